# revision 2
# baseline (speedup 1.0000x reference)
"""GatedSparseAttention on 8 Trainium2 NeuronCores (Bass/Tile).

Sequence-parallel over query blocks: each core owns QB = T/8 query rows.
Inputs are sharded 8-way on the host and reconstructed on device by one
AllGather per dtype pack; K/V/indexer-key projections are computed redundantly
per core.  Top-k selection is a per-row threshold found by bisection (ACT Sign
+ row accumulate); attention is dense-masked, which equals gather-based top-k
attention up to boundary ties.  The indexer-score path runs in f32 (selection
is sensitive to score perturbation); the main attention path runs in bf16.

The Bass program is built, compiled and warmed up at import time; kernel()
only packs inputs, runs the SPMD NEFF via PJRT, and unpacks the output.
"""

import math
import sys

sys.path.insert(0, "/opt/trn_rl_repo")

import numpy as np
import ml_dtypes

import concourse.bass as bass
import concourse.mybir as mybir
from concourse.tile import TileContext
from concourse.bass_utils import run_bass_kernel_spmd

bf16 = mybir.dt.bfloat16
f32 = mybir.dt.float32
AF = mybir.ActivationFunctionType
ALU = mybir.AluOpType

T = 2048
D = 512
H = 8
DH = 64
NIDX = 4
DIDX = 64
KSEL = 128
BIG = 30.0
N_BISECT = 21
N_CORES = 8

W_NAMES = ["Wq", "Wk", "Wv", "Wo", "Wvg", "Wog"]
AUX_LEN = 1040  # bvg 512 | bog 512 | biw 4 | idx_bias 4 | q0 1 | pad


def layout16(T, n_cores):
    QB = T // n_cores
    offs = {}
    o = 0
    for w in W_NAMES:
        offs[w] = (o, D // n_cores, D)
        o += (D // n_cores) * D
    offs["cosb"] = (o, 128, QB)
    o += 128 * QB
    offs["nsinb"] = (o, 128, QB)
    o += 128 * QB
    return offs, o


def layout32(T, n_cores):
    QB = T // n_cores
    offs = {"xTblk": (0, D, QB)}
    o = D * QB
    for name, w in (("Wiq", NIDX * DIDX), ("Wik", DIDX), ("Wiw", NIDX)):
        offs[name] = (o, D // n_cores, w)
        o += (D // n_cores) * w
    return offs, o


def make_psg_flip():
    B = np.zeros((64, 64), np.float32)
    for d in range(32):
        B[d, d + 32] = 1.0
        B[d + 32, d] = -1.0
    P = np.zeros((128, 128), np.float32)
    P[0:64, 0:64] = B
    P[64:128, 64:128] = B
    return P.T.astype(ml_dtypes.bfloat16)


def make_rope_tables(T):
    invf = 1.0 / (10000.0 ** (np.arange(0, DH, 2, dtype=np.float64) / DH))
    invcol = np.concatenate([invf, invf, invf, invf]).reshape(128, 1)
    ph = invcol * np.arange(T, dtype=np.float64)[None, :]
    return np.cos(ph).astype(np.float32), (-np.sin(ph)).astype(np.float32)


def build_nc(T=T, n_cores=N_CORES):
    NJB = T // 512
    NT = T // 128
    QB = T // n_cores
    NCH = QB // 128
    CPJ = 512 // QB if QB < 512 else 1
    offs16, N16 = layout16(T, n_cores)
    offs32, N32 = layout32(T, n_cores)

    nc = bass.Bass(num_devices=n_cores)
    pk16 = nc.declare_dram_parameter("pk16", [N16], bf16, isOutput=False)
    pk32 = nc.declare_dram_parameter("pk32", [N32], f32, isOutput=False)
    aux32 = nc.declare_dram_parameter("aux32", [AUX_LEN], f32, isOutput=False)
    yout = nc.declare_dram_parameter("yout", [D, QB], bf16, isOutput=True)

    psg_c = nc.inline_tensor(make_psg_flip(), name="psgT_flip")
    eye_c = nc.inline_tensor(np.eye(128, dtype=ml_dtypes.bfloat16), name="eye128")

    from contextlib import ExitStack

    with TileContext(nc) as tc, ExitStack() as ctx:
        sb = ctx.enter_context(tc.tile_pool(name="sb", bufs=1))
        dram = ctx.enter_context(tc.tile_pool(name="dram", bufs=1, space="DRAM"))
        ps = ctx.enter_context(tc.tile_pool(name="ps", bufs=3, space="PSUM"))

        # ---------------- gather ----------------
        bounce16 = dram.tile([N16], bf16)
        g16t = dram.tile([n_cores * N16], bf16, addr_space="Shared")
        bounce32 = dram.tile([N32], f32)
        g32t = dram.tile([n_cores * N32], f32, addr_space="Shared")
        nc.sync.dma_start(bounce16[:], pk16[:])
        nc.sync.dma_start(bounce32[:], pk32[:])
        grp = [list(range(n_cores))]
        nc.gpsimd.collective_compute(
            "AllGather", ALU.bypass, replica_groups=grp,
            ins=[bounce16[:]], outs=[g16t[:]],
        )
        nc.gpsimd.collective_compute(
            "AllGather", ALU.bypass, replica_groups=grp,
            ins=[bounce32[:]], outs=[g32t[:]],
        )
        g16 = g16t.rearrange("(c n) -> c n", c=n_cores)
        g32 = g32t.rearrange("(c n) -> c n", c=n_cores)

        def load_rowshard(dest, gview, offmap, name, row0):
            off, r, w = offmap[name]
            c0 = row0 // r
            cnt = 128 // r
            src = gview[c0 : c0 + cnt, off : off + r * w].rearrange(
                "c (r w) -> c r w", w=w
            )
            nc.sync.dma_start(dest, src)

        # ---------------- SBUF loads ----------------
        Wsb = {}
        for wname in W_NAMES:
            Wsb[wname] = [
                sb.tile([128, D], bf16, name=f"{wname}_{k}") for k in range(4)
            ]
            for k in range(4):
                load_rowshard(Wsb[wname][k][:], g16, offs16, wname, 128 * k)
        Wiq_sb = [sb.tile([128, NIDX * DIDX], f32, name=f"Wiq{k}") for k in range(4)]
        Wik_sb = [sb.tile([128, DIDX], f32, name=f"Wik{k}") for k in range(4)]
        Wiw_sb = [sb.tile([128, NIDX], f32, name=f"Wiw{k}") for k in range(4)]
        for k in range(4):
            load_rowshard(Wiq_sb[k][:], g32, offs32, "Wiq", 128 * k)
            load_rowshard(Wik_sb[k][:], g32, offs32, "Wik", 128 * k)
            load_rowshard(Wiw_sb[k][:], g32, offs32, "Wiw", 128 * k)

        aux = sb.tile([1, AUX_LEN], f32)
        nc.sync.dma_start(aux[:], aux32.rearrange("(o n) -> o n", o=1))
        bogT = [sb.tile([128, 1], f32, name=f"bogT{k}") for k in range(4)]
        for k in range(4):
            nc.sync.dma_start(
                bogT[k][:],
                aux32[512 + 128 * k : 512 + 128 * (k + 1)].rearrange(
                    "(p o) -> p o", o=1
                ),
            )
        psgT = sb.tile([128, 128], bf16)
        nc.sync.dma_start(psgT[:], psg_c[:])
        eye = sb.tile([128, 128], bf16)
        nc.sync.dma_start(eye[:], eye_c[:])

        # ---------------- broadcasts (ones matmul, K=1) ----------------
        ones32 = sb.tile([1, 128], f32)
        nc.vector.memset(ones32[:], 1.0)

        def bcast(src_ap, width, name):
            p_ = ps.tile([128, 512], f32, tag="mm", name=f"bc_{name}")
            nc.tensor.matmul(
                p_[0:128, 0:width], ones32[:], src_ap, start=True, stop=True
            )
            out = sb.tile([128, width], f32, name=name)
            nc.scalar.copy(out[:], p_[0:128, 0:width])
            return out

        bvg_bc = bcast(aux[0:1, 0:512], 512, "bvg_bc")
        idxb_bc = bcast(aux[0:1, 1028:1032], NIDX, "idxb_bc")
        biw_bc = bcast(aux[0:1, 1024:1028], NIDX, "biw_bc")
        q0_bc = bcast(aux[0:1, 1032:1033], 1, "q0_bc")

        # ---------------- ramps ----------------
        jramp = sb.tile([128, T], f32)
        nc.gpsimd.iota(
            jramp[:], pattern=[[1, T]], base=0, channel_multiplier=0,
            allow_small_or_imprecise_dtypes=True,
        )
        pramp = sb.tile([128, 1], f32)
        nc.gpsimd.iota(
            pramp[:], pattern=[[0, 1]], base=0, channel_multiplier=1,
            allow_small_or_imprecise_dtypes=True,
        )
        rowid = []
        for c in range(NCH):
            rid = sb.tile([128, 1], f32, name=f"rowid{c}")
            nc.vector.tensor_scalar(rid[:], pramp[:], q0_bc[:, 0:1], None, op0=ALU.add)
            if c:
                nc.vector.tensor_scalar(
                    rid[:], rid[:], float(128 * c), None, op0=ALU.add
                )
            rowid.append(rid)

        # rope tables: full-T from gathered blocks; local q tables from param
        cosk = sb.tile([128, T], bf16)
        nsink = sb.tile([128, T], bf16)
        cosq = sb.tile([128, QB], bf16)
        nsinq = sb.tile([128, QB], bf16)
        oc16, _, _ = offs16["cosb"]
        os16, _, _ = offs16["nsinb"]
        for c in range(n_cores):
            for off_, dst in ((oc16, cosk), (os16, nsink)):
                src = g16[c, off_ : off_ + 128 * QB].rearrange("(r w) -> r w", w=QB)
                nc.sync.dma_start(dst[:, c * QB : (c + 1) * QB], src)
        for off_, dst in ((oc16, cosq), (os16, nsinq)):
            src = pk16[off_ : off_ + 128 * QB].rearrange("(r w) -> r w", w=QB)
            nc.sync.dma_start(dst[:], src)

        negbig = sb.tile([128, 1], f32)
        nc.vector.memset(negbig[:], -BIG)

        # ------- x staging: f32 for indexer, cast to bf16 for main path -------
        tmpp = ctx.enter_context(tc.tile_pool(name="tmpp", bufs=3))
        xT16 = [sb.tile([128, T], bf16, name=f"xT16_{k}") for k in range(4)]
        xb16 = [sb.tile([128, QB], bf16, name=f"xb16_{k}") for k in range(4)]
        kiT = sb.tile([128, T], f32)  # duplicated halves
        qiT = [sb.tile([128, QB], f32, name=f"qiT{m}") for m in range(2)]
        wsig = [sb.tile([128, NIDX], f32, name=f"wsig{c}") for c in range(NCH)]
        oxb, _, _ = offs32["xTblk"]

        with tc.tile_pool(name="xf", bufs=2) as xf:
            xbf = []
            for k in range(4):
                t_ = xf.tile([128, QB], f32, tag=f"xbf{k}", name=f"xbf{k}", bufs=1)
                src = pk32[oxb + 128 * k * QB : oxb + 128 * (k + 1) * QB].rearrange(
                    "(r w) -> r w", w=QB
                )
                nc.sync.dma_start(t_[:], src)
                nc.vector.tensor_copy(xb16[k][:], t_[:])
                xbf.append(t_)

            for m in range(2):
                pq = ps.tile([128, 512], f32, tag="mm", name="qi_ps")
                for k in range(4):
                    nc.tensor.matmul(
                        pq[:, 0:QB],
                        Wiq_sb[k][:, 128 * m : 128 * (m + 1)],
                        xbf[k][:],
                        start=(k == 0),
                        stop=(k == 3),
                    )
                nc.scalar.copy(qiT[m][:], pq[:, 0:QB])

            for c in range(NCH):
                csl = slice(128 * c, 128 * (c + 1))
                pw = ps.tile([128, 512], f32, tag="mm", name="w_ps")
                for k in range(4):
                    nc.tensor.matmul(
                        pw[:, 0:NIDX], xbf[k][:, csl], Wiw_sb[k][:],
                        start=(k == 0), stop=(k == 3),
                    )
                nc.vector.tensor_tensor(
                    wsig[c][:], pw[:, 0:NIDX], biw_bc[:], op=ALU.add
                )
                nc.scalar.activation(wsig[c][:], wsig[c][:], AF.Sigmoid)

            for jb in range(NJB):
                jsl = slice(512 * jb, 512 * (jb + 1))
                xft = []
                for k in range(4):
                    t_ = xf.tile([128, 512], f32, tag="xft", name=f"xft{k}_{jb}")
                    for ci in range(CPJ):
                        c = (512 * jb) // QB + ci
                        src = g32[
                            c, oxb + 128 * k * QB : oxb + 128 * (k + 1) * QB
                        ].rearrange("(r w) -> r w", w=QB)
                        nc.sync.dma_start(t_[:, ci * QB : (ci + 1) * QB], src)
                    nc.vector.tensor_copy(xT16[k][:, jsl], t_[:])
                    xft.append(t_)
                pki = ps.tile([128, 512], f32, tag="mm", name="ki_ps")
                for k in range(4):
                    nc.tensor.matmul(
                        pki[0:64, :], Wik_sb[k][:], xft[k][:],
                        start=(k == 0), stop=(k == 3),
                    )
                nc.scalar.copy(kiT[0:64, jsl], pki[0:64, :])
                nc.sync.dma_start(kiT[64:128, jsl], kiT[0:64, jsl])

        # ---------------- projections (bf16 main path) ----------------
        def mm_acc(out_ps, lhsT_list, rhs_list):
            n = len(lhsT_list)
            for k in range(n):
                nc.tensor.matmul(
                    out_ps, lhsT_list[k], rhs_list[k],
                    start=(k == 0), stop=(k == n - 1),
                )

        def rope_apply(raw_bf, cos_t, nsin_t, out_bf, width):
            rot = ps.tile([128, 512], f32, tag="mm", name="rot_ps")
            nc.tensor.matmul(rot[:, 0:width], psgT[:], raw_bf, start=True, stop=True)
            t1 = tmpp.tile([128, 512], f32, tag="ropet", name="rope_t1")
            nc.vector.tensor_tensor(t1[:, 0:width], raw_bf, cos_t, op=ALU.mult)
            t2 = tmpp.tile([128, 512], f32, tag="ropet", name="rope_t2")
            nc.vector.tensor_tensor(
                t2[:, 0:width], rot[:, 0:width], nsin_t, op=ALU.mult
            )
            nc.vector.tensor_tensor(out_bf, t1[:, 0:width], t2[:, 0:width], op=ALU.add)

        kTr = [sb.tile([128, T], bf16, name=f"kTr{m}") for m in range(4)]
        for m in range(4):
            for jb in range(NJB):
                jsl = slice(512 * jb, 512 * (jb + 1))
                pk_ = ps.tile([128, 512], f32, tag="mm", name="kT_ps")
                mm_acc(
                    pk_[:],
                    [Wsb["Wk"][k][:, 128 * m : 128 * (m + 1)] for k in range(4)],
                    [xT16[k][:, jsl] for k in range(4)],
                )
                raw = tmpp.tile([128, 512], bf16, tag="raw16", name="kT_raw")
                nc.scalar.copy(raw[:], pk_[:])
                rope_apply(raw[:], cosk[:, jsl], nsink[:, jsl], kTr[m][:, jsl], 512)

        v_nat = [sb.tile([128, D], bf16, name=f"v{t}") for t in range(NT)]
        for t in range(NT):
            tsl = slice(128 * t, 128 * (t + 1))
            pv = ps.tile([128, 512], f32, tag="mm", name="v_ps")
            mm_acc(pv[:], [xT16[k][:, tsl] for k in range(4)],
                   [Wsb["Wv"][k][:] for k in range(4)])
            pg = ps.tile([128, 512], f32, tag="mm", name="vg_ps")
            mm_acc(pg[:], [xT16[k][:, tsl] for k in range(4)],
                   [Wsb["Wvg"][k][:] for k in range(4)])
            gs = tmpp.tile([128, D], f32, tag="vgs", name="vg_sig")
            nc.vector.tensor_tensor(gs[:], pg[:], bvg_bc[:], op=ALU.add)
            nc.scalar.activation(gs[:], gs[:], AF.Sigmoid)
            nc.vector.tensor_tensor(v_nat[t][:], pv[:], gs[:], op=ALU.mult)

        qTr = [sb.tile([128, QB], bf16, name=f"qTr{m}") for m in range(4)]
        for m in range(4):
            pq = ps.tile([128, 512], f32, tag="mm", name="qT_ps")
            mm_acc(pq[:, 0:QB],
                   [Wsb["Wq"][k][:, 128 * m : 128 * (m + 1)] for k in range(4)],
                   [xb16[k][:] for k in range(4)])
            raw = tmpp.tile([128, 512], bf16, tag="raw16", name="qT_raw")
            nc.scalar.activation(
                raw[:, 0:QB], pq[:, 0:QB], AF.Copy, scale=float(DH ** -0.5)
            )
            rope_apply(raw[:, 0:QB], cosq[:], nsinq[:], qTr[m][:], QB)

        ogT = [sb.tile([128, QB], bf16, name=f"ogT{m}") for m in range(4)]
        for m in range(4):
            pg = ps.tile([128, 512], f32, tag="mm", name="og_ps")
            mm_acc(pg[:, 0:QB],
                   [Wsb["Wog"][k][:, 128 * m : 128 * (m + 1)] for k in range(4)],
                   [xb16[k][:] for k in range(4)])
            nc.scalar.activation(
                ogT[m][:], pg[:, 0:QB], AF.Sigmoid, bias=bogT[m][:, 0:1]
            )

        # ---------------- indexer scores + selection ----------------
        fs = [sb.tile([128, T], f32, name=f"fs{c}") for c in range(NCH)]
        m01 = [sb.tile([128, T], f32, name=f"m01_{c}") for c in range(NCH)]
        for c in range(NCH):
            csl = slice(128 * c, 128 * (c + 1))
            for jb in range(NJB):
                jsl = slice(512 * jb, 512 * (jb + 1))
                for h in range(NIDX):
                    pr = ps.tile([128, 512], f32, tag="mm", name="raw_ps")
                    hr = slice(64 * (h % 2), 64 * (h % 2) + 64)
                    nc.tensor.matmul(
                        pr[:], qiT[h // 2][hr, csl], kiT[hr, jsl],
                        start=True, stop=True,
                    )
                    g = tmpp.tile([128, 512], f32, tag="gsig", name="g_sig")
                    nc.scalar.activation(
                        g[:], pr[:], AF.Sigmoid,
                        bias=idxb_bc[:, h : h + 1],
                        scale=float(1.0 / math.sqrt(DIDX)),
                    )
                    if h == 0:
                        nc.vector.tensor_scalar(
                            fs[c][:, jsl], g[:], wsig[c][:, 0:1], 1.0,
                            op0=ALU.mult, op1=ALU.add,
                        )
                    else:
                        nc.vector.scalar_tensor_tensor(
                            fs[c][:, jsl], in0=g[:], scalar=wsig[c][:, h : h + 1],
                            in1=fs[c][:, jsl], op0=ALU.mult, op1=ALU.add,
                        )
                nc.vector.scalar_tensor_tensor(
                    fs[c][:, jsl], in0=jramp[:, jsl], scalar=rowid[c][:, 0:1],
                    in1=fs[c][:, jsl], op0=ALU.is_le, op1=ALU.mult,
                )

            nlo = sb.tile([128, 1], f32, name=f"nlo{c}")
            nhi = sb.tile([128, 1], f32, name=f"nhi{c}")
            nc.vector.memset(nlo[:], -5.5)
            nc.vector.memset(nhi[:], -0.5)
            nmid = sb.tile([128, 1], f32, name=f"nmid{c}")
            S = sb.tile([128, 1], f32, name=f"S{c}")
            cfl = sb.tile([128, 1], f32, name=f"cfl{c}")
            cflm = sb.tile([128, 1], f32, name=f"cflm{c}")
            dd = sb.tile([128, 1], f32, name=f"dd{c}")
            thr = float(2 * KSEL - T)
            for it in range(N_BISECT):
                sgn_scr = tmpp.tile([128, T], bf16, tag="sgn", name="sgn_scr", bufs=1)
                nc.vector.tensor_tensor(nmid[:], nlo[:], nhi[:], op=ALU.add)
                nc.vector.tensor_scalar(nmid[:], nmid[:], 0.5, None, op0=ALU.mult)
                nc.scalar.activation(
                    sgn_scr[:], fs[c][:], AF.Sign, bias=nmid[:, 0:1], accum_out=S[:]
                )
                nc.vector.tensor_scalar(cfl[:], S[:], thr, None, op0=ALU.is_ge)
                nc.vector.tensor_scalar(
                    cflm[:], cfl[:], -1.0, 1.0, op0=ALU.mult, op1=ALU.add
                )
                nc.vector.tensor_tensor(dd[:], nmid[:], nhi[:], op=ALU.subtract)
                nc.vector.scalar_tensor_tensor(
                    nhi[:], in0=dd[:], scalar=cfl[:, 0:1], in1=nhi[:],
                    op0=ALU.mult, op1=ALU.add,
                )
                nc.vector.tensor_tensor(dd[:], nmid[:], nlo[:], op=ALU.subtract)
                nc.vector.scalar_tensor_tensor(
                    nlo[:], in0=dd[:], scalar=cflm[:, 0:1], in1=nlo[:],
                    op0=ALU.mult, op1=ALU.add,
                )
            tau = sb.tile([128, 1], f32, name=f"tau{c}")
            nc.vector.tensor_scalar(tau[:], nhi[:], -1.0, None, op0=ALU.mult)
            nc.vector.tensor_scalar(
                m01[c][:], fs[c][:], tau[:, 0:1], None, op0=ALU.is_ge
            )

        # ---------------- attention ----------------
        psT = ctx.enter_context(tc.tile_pool(name="psT", bufs=2, space="PSUM"))
        psA = ctx.enter_context(tc.tile_pool(name="psA", bufs=2, space="PSUM"))
        pp = ctx.enter_context(tc.tile_pool(name="pp", bufs=2))

        attn_g = []
        for hp in range(4):
            attn_ps = psA.tile([128, QB], f32, tag="attn", name=f"attn{hp}")
            for h in (2 * hp, 2 * hp + 1):
                hrow = slice(64 * (h % 2), 64 * (h % 2) + 64)
                pT = pp.tile(
                    [128, NT * NCH * 128], bf16, tag="pT", name=f"pT{h}", bufs=1
                )
                for c in range(NCH):
                    csl = slice(128 * c, 128 * (c + 1))
                    p_t = pp.tile([128, T], bf16, tag="p", name=f"p{h}_{c}")
                    den4 = pp.tile(
                        [128, NJB], f32, tag="den", name=f"den4_{h}_{c}", bufs=4
                    )
                    for jb in range(NJB):
                        jsl = slice(512 * jb, 512 * (jb + 1))
                        sps_ = ps.tile([128, 512], f32, tag="mm", name="s_ps")
                        nc.tensor.matmul(
                            sps_[:], qTr[h // 2][hrow, csl], kTr[h // 2][hrow, jsl],
                            start=True, stop=True,
                        )
                        nc.vector.scalar_tensor_tensor(
                            sps_[:], in0=m01[c][:, jsl], scalar=BIG, in1=sps_[:],
                            op0=ALU.mult, op1=ALU.add,
                        )
                        nc.scalar.activation(
                            p_t[:, jsl], sps_[:], AF.Exp,
                            bias=negbig[:, 0:1], accum_out=den4[:, jb : jb + 1],
                        )
                    den = pp.tile(
                        [128, 1], f32, tag="den1", name=f"den_{h}_{c}", bufs=4
                    )
                    nc.vector.reduce_sum(den[:], den4[:], axis=mybir.AxisListType.X)
                    inv = pp.tile(
                        [128, 1], f32, tag="inv", name=f"inv_{h}_{c}", bufs=4
                    )
                    nc.vector.reciprocal(inv[:], den[:])
                    nc.vector.tensor_scalar(
                        p_t[:], p_t[:], inv[:, 0:1], None, op0=ALU.mult
                    )
                    for t in range(NT):
                        trp = psT.tile([128, 128], bf16, tag="tr", name="tr_ps")
                        nc.tensor.transpose(
                            trp[:], p_t[:, 128 * t : 128 * (t + 1)], eye[:]
                        )
                        nc.vector.tensor_copy(
                            pT[:, t * NCH * 128 + 128 * c
                               : t * NCH * 128 + 128 * (c + 1)],
                            trp[:],
                        )
                for t in range(NT):
                    nc.tensor.matmul(
                        attn_ps[hrow, :],
                        v_nat[t][:, 64 * h : 64 * h + 64],
                        pT[:, t * NCH * 128 : (t + 1) * NCH * 128],
                        start=(t == 0),
                        stop=(t == NT - 1),
                    )
            ag = sb.tile([128, QB], bf16, name=f"attn_g{hp}")
            nc.vector.tensor_tensor(ag[:], attn_ps[:], ogT[hp][:], op=ALU.mult)
            attn_g.append(ag)

        for m in range(4):
            py = ps.tile([128, 512], f32, tag="mm", name="y_ps")
            mm_acc(py[:, 0:QB],
                   [Wsb["Wo"][k][:, 128 * m : 128 * (m + 1)] for k in range(4)],
                   [attn_g[k][:] for k in range(4)])
            ysb = tmpp.tile([128, QB], bf16, tag="ysb", name="y_sb")
            nc.scalar.copy(ysb[:], py[:, 0:QB])
            nc.sync.dma_start(yout[128 * m : 128 * (m + 1), :], ysb[:])

    return nc


def split_multi_waits(nc, max_waits=1):
    """Walrus here rejects instructions with more than one sync wait; split
    extras into single-wait NoOps on the same engine (in-order execution
    makes this semantics-preserving)."""
    n_extra = 0
    for f in nc.m.functions:
        for bb in f.blocks:
            insts = list(bb.instructions)
            out = []
            changed = False
            for inst in insts:
                si = inst.sync_info
                waits = list(si.on_wait) if si is not None else []
                if len(waits) > max_waits:
                    head, keep = waits[:-max_waits], waits[-max_waits:]
                    for w in head:
                        n_extra += 1
                        out.append(
                            mybir.InstNoOp(
                                name=f"I-waitsplit-{n_extra}",
                                engine=inst.engine,
                                sync_info=mybir.SyncInfo(on_wait=[w], on_update=[]),
                            )
                        )
                    inst.sync_info = mybir.SyncInfo(
                        on_wait=keep, on_update=list(si.on_update)
                    )
                    changed = True
                out.append(inst)
            if changed:
                bb.instructions = out
    return n_extra


def pack_inputs(inputs, T=T, n_cores=N_CORES):
    offs16, N16 = layout16(T, n_cores)
    offs32, N32 = layout32(T, n_cores)
    QB = T // n_cores
    x = np.asarray(inputs["x"], np.float32).reshape(T, D)
    xT = np.ascontiguousarray(x.T)
    rs = D // n_cores
    cosf, nsinf = make_rope_tables(T)
    cosf = cosf.astype(ml_dtypes.bfloat16)
    nsinf = nsinf.astype(ml_dtypes.bfloat16)

    in_maps = []
    for c in range(n_cores):
        pkb = np.empty(N16, dtype=ml_dtypes.bfloat16)
        for wn in W_NAMES:
            o, r, w = offs16[wn]
            pkb[o : o + r * w] = (
                np.asarray(inputs[wn], np.float32)[c * rs : (c + 1) * rs]
                .astype(ml_dtypes.bfloat16)
                .ravel()
            )
        o, r, w = offs16["cosb"]
        pkb[o : o + r * w] = cosf[:, c * QB : (c + 1) * QB].ravel()
        o, r, w = offs16["nsinb"]
        pkb[o : o + r * w] = nsinf[:, c * QB : (c + 1) * QB].ravel()

        pkf = np.empty(N32, dtype=np.float32)
        o, r, w = offs32["xTblk"]
        pkf[o : o + r * w] = xT[:, c * QB : (c + 1) * QB].ravel()
        for wn in ("Wiq", "Wik", "Wiw"):
            o, r, w = offs32[wn]
            pkf[o : o + r * w] = np.asarray(inputs[wn], np.float32)[
                c * rs : (c + 1) * rs
            ].ravel()

        aux = np.zeros(AUX_LEN, np.float32)
        aux[0:512] = np.asarray(inputs["bvg"], np.float32)
        aux[512:1024] = np.asarray(inputs["bog"], np.float32)
        aux[1024:1028] = np.asarray(inputs["biw"], np.float32)
        aux[1028:1032] = np.asarray(inputs["idx_bias"], np.float32)
        aux[1032] = float(c * QB)
        in_maps.append({"pk16": pkb, "pk32": pkf, "aux32": aux})
    return in_maps


def unpack_output(results, T=T, n_cores=N_CORES):
    QB = T // n_cores
    out = np.empty((T, D), np.float32)
    for c in range(n_cores):
        yT = np.asarray(results[c]["yout"]).astype(np.float32)
        out[c * QB : (c + 1) * QB, :] = yT.T
    return out.reshape(1, T, D)


# ---------------- import-time build + compile + warm-up ----------------
_nc = build_nc()
split_multi_waits(_nc)

_dummy = {
    "x": np.zeros((1, T, D), np.float32),
    "Wq": np.zeros((D, D), np.float32),
    "Wk": np.zeros((D, D), np.float32),
    "Wv": np.zeros((D, D), np.float32),
    "Wo": np.zeros((D, D), np.float32),
    "Wiq": np.zeros((D, NIDX * DIDX), np.float32),
    "Wik": np.zeros((D, DIDX), np.float32),
    "Wiw": np.zeros((D, NIDX), np.float32),
    "biw": np.zeros((NIDX,), np.float32),
    "idx_bias": np.zeros((NIDX,), np.float32),
    "Wvg": np.zeros((D, D), np.float32),
    "bvg": np.zeros((D,), np.float32),
    "Wog": np.zeros((D, D), np.float32),
    "bog": np.zeros((D,), np.float32),
}
run_bass_kernel_spmd(_nc, pack_inputs(_dummy), list(range(N_CORES)))


def kernel(**inputs):
    in_maps = pack_inputs(inputs)
    r = run_bass_kernel_spmd(_nc, in_maps, list(range(N_CORES)))
    return unpack_output(r.results)


# revision 3
# speedup vs baseline: 2.4450x; 2.4450x over previous
"""GatedSparseAttention on 8 Trainium2 NeuronCores (Bass/Tile).

Sequence-parallel over query blocks: each core owns QB = T/8 query rows.
Inputs are sharded 8-way on the host and reconstructed on device by one
AllGather per dtype pack; K/V/indexer-key projections are computed redundantly
per core.  Top-k selection is a per-row threshold found by bisection (ACT Sign
+ row accumulate); attention is dense-masked, which equals gather-based top-k
attention up to boundary ties.  The indexer-score path runs in f32 (selection
is sensitive to score perturbation); the main attention path runs in bf16.

The Bass program is built, compiled and warmed up at import time; kernel()
only packs inputs, runs the SPMD NEFF via PJRT, and unpacks the output.
"""

import math
import sys

sys.path.insert(0, "/opt/trn_rl_repo")

import numpy as np
import ml_dtypes

import concourse.bass as bass
import concourse.mybir as mybir
from concourse.tile import TileContext
from concourse.bass_utils import run_bass_kernel_spmd

bf16 = mybir.dt.bfloat16
f32 = mybir.dt.float32
AF = mybir.ActivationFunctionType
ALU = mybir.AluOpType

T = 2048
D = 512
H = 8
DH = 64
NIDX = 4
DIDX = 64
KSEL = 128
BIG = 30.0
N_BISECT = 21
N_CORES = 8

W_NAMES = ["Wq", "Wk", "Wv", "Wo", "Wvg", "Wog"]
AUX_LEN = 1040  # bvg 512 | bog 512 | biw 4 | idx_bias 4 | q0 1 | pad


def layout16(T, n_cores):
    QB = T // n_cores
    offs = {}
    o = 0
    for w in W_NAMES:
        offs[w] = (o, D // n_cores, D)
        o += (D // n_cores) * D
    offs["cosb"] = (o, 128, QB)
    o += 128 * QB
    offs["nsinb"] = (o, 128, QB)
    o += 128 * QB
    return offs, o


def layout32(T, n_cores):
    QB = T // n_cores
    offs = {"xTblk": (0, D, QB)}
    o = D * QB
    for name, w in (("Wiq", NIDX * DIDX), ("Wik", DIDX), ("Wiw", NIDX)):
        offs[name] = (o, D // n_cores, w)
        o += (D // n_cores) * w
    return offs, o


def make_psg_flip():
    B = np.zeros((64, 64), np.float32)
    for d in range(32):
        B[d, d + 32] = 1.0
        B[d + 32, d] = -1.0
    P = np.zeros((128, 128), np.float32)
    P[0:64, 0:64] = B
    P[64:128, 64:128] = B
    return P.T.astype(ml_dtypes.bfloat16)


def make_rope_tables(T):
    invf = 1.0 / (10000.0 ** (np.arange(0, DH, 2, dtype=np.float64) / DH))
    invcol = np.concatenate([invf, invf, invf, invf]).reshape(128, 1)
    ph = invcol * np.arange(T, dtype=np.float64)[None, :]
    return np.cos(ph).astype(np.float32), (-np.sin(ph)).astype(np.float32)


def build_nc(T=T, n_cores=N_CORES):
    NJB = T // 512
    NT = T // 128
    QB = T // n_cores
    NCH = QB // 128
    CPJ = 512 // QB if QB < 512 else 1
    offs16, N16 = layout16(T, n_cores)
    offs32, N32 = layout32(T, n_cores)

    nc = bass.Bass(num_devices=n_cores)
    pk16 = nc.declare_dram_parameter("pk16", [N16], bf16, isOutput=False)
    pk32 = nc.declare_dram_parameter("pk32", [N32], f32, isOutput=False)
    aux32 = nc.declare_dram_parameter("aux32", [AUX_LEN], f32, isOutput=False)
    yout = nc.declare_dram_parameter("yout", [D, QB], bf16, isOutput=True)

    psg_c = nc.inline_tensor(make_psg_flip(), name="psgT_flip")
    eye_c = nc.inline_tensor(np.eye(128, dtype=ml_dtypes.bfloat16), name="eye128")

    from contextlib import ExitStack

    with TileContext(nc) as tc, ExitStack() as ctx:
        sb = ctx.enter_context(tc.tile_pool(name="sb", bufs=1))
        dram = ctx.enter_context(tc.tile_pool(name="dram", bufs=1, space="DRAM"))
        ps = ctx.enter_context(tc.tile_pool(name="ps", bufs=3, space="PSUM"))

        # ---------------- gather ----------------
        bounce16 = dram.tile([N16], bf16)
        g16t = dram.tile([n_cores * N16], bf16, addr_space="Shared")
        bounce32 = dram.tile([N32], f32)
        g32t = dram.tile([n_cores * N32], f32, addr_space="Shared")
        nc.sync.dma_start(bounce16[:], pk16[:])
        nc.sync.dma_start(bounce32[:], pk32[:])
        grp = [list(range(n_cores))]
        nc.gpsimd.collective_compute(
            "AllGather", ALU.bypass, replica_groups=grp,
            ins=[bounce16[:]], outs=[g16t[:]],
        )
        nc.gpsimd.collective_compute(
            "AllGather", ALU.bypass, replica_groups=grp,
            ins=[bounce32[:]], outs=[g32t[:]],
        )
        g16 = g16t.rearrange("(c n) -> c n", c=n_cores)
        g32 = g32t.rearrange("(c n) -> c n", c=n_cores)

        def load_rowshard(dest, gview, offmap, name, row0):
            off, r, w = offmap[name]
            c0 = row0 // r
            cnt = 128 // r
            src = gview[c0 : c0 + cnt, off : off + r * w].rearrange(
                "c (r w) -> c r w", w=w
            )
            nc.sync.dma_start(dest, src)

        # ---------------- SBUF loads ----------------
        Wsb = {}
        for wname in W_NAMES:
            Wsb[wname] = [
                sb.tile([128, D], bf16, name=f"{wname}_{k}") for k in range(4)
            ]
            for k in range(4):
                load_rowshard(Wsb[wname][k][:], g16, offs16, wname, 128 * k)
        Wiq_sb = [sb.tile([128, NIDX * DIDX], f32, name=f"Wiq{k}") for k in range(4)]
        Wik_sb = [sb.tile([128, DIDX], f32, name=f"Wik{k}") for k in range(4)]
        Wiw_sb = [sb.tile([128, NIDX], f32, name=f"Wiw{k}") for k in range(4)]
        for k in range(4):
            load_rowshard(Wiq_sb[k][:], g32, offs32, "Wiq", 128 * k)
            load_rowshard(Wik_sb[k][:], g32, offs32, "Wik", 128 * k)
            load_rowshard(Wiw_sb[k][:], g32, offs32, "Wiw", 128 * k)

        aux = sb.tile([1, AUX_LEN], f32)
        nc.sync.dma_start(aux[:], aux32.rearrange("(o n) -> o n", o=1))
        bogT = [sb.tile([128, 1], f32, name=f"bogT{k}") for k in range(4)]
        for k in range(4):
            nc.sync.dma_start(
                bogT[k][:],
                aux32[512 + 128 * k : 512 + 128 * (k + 1)].rearrange(
                    "(p o) -> p o", o=1
                ),
            )
        psgT = sb.tile([128, 128], bf16)
        nc.sync.dma_start(psgT[:], psg_c[:])
        eye = sb.tile([128, 128], bf16)
        nc.sync.dma_start(eye[:], eye_c[:])

        # ---------------- broadcasts (ones matmul, K=1) ----------------
        ones32 = sb.tile([1, 128], f32)
        nc.vector.memset(ones32[:], 1.0)

        def bcast(src_ap, width, name):
            p_ = ps.tile([128, 512], f32, tag="mm", name=f"bc_{name}")
            nc.tensor.matmul(
                p_[0:128, 0:width], ones32[:], src_ap, start=True, stop=True
            )
            out = sb.tile([128, width], f32, name=name)
            nc.scalar.copy(out[:], p_[0:128, 0:width])
            return out

        bvg_bc = bcast(aux[0:1, 0:512], 512, "bvg_bc")
        idxb_bc = bcast(aux[0:1, 1028:1032], NIDX, "idxb_bc")
        biw_bc = bcast(aux[0:1, 1024:1028], NIDX, "biw_bc")
        q0_bc = bcast(aux[0:1, 1032:1033], 1, "q0_bc")

        # ---------------- ramps ----------------
        jramp = sb.tile([128, T], f32)
        nc.gpsimd.iota(
            jramp[:], pattern=[[1, T]], base=0, channel_multiplier=0,
            allow_small_or_imprecise_dtypes=True,
        )
        pramp = sb.tile([128, 1], f32)
        nc.gpsimd.iota(
            pramp[:], pattern=[[0, 1]], base=0, channel_multiplier=1,
            allow_small_or_imprecise_dtypes=True,
        )
        rowid = []
        for c in range(NCH):
            rid = sb.tile([128, 1], f32, name=f"rowid{c}")
            nc.vector.tensor_scalar(rid[:], pramp[:], q0_bc[:, 0:1], None, op0=ALU.add)
            if c:
                nc.vector.tensor_scalar(
                    rid[:], rid[:], float(128 * c), None, op0=ALU.add
                )
            rowid.append(rid)

        # rope tables: full-T from gathered blocks; local q tables from param
        cosk = sb.tile([128, T], bf16)
        nsink = sb.tile([128, T], bf16)
        cosq = sb.tile([128, QB], bf16)
        nsinq = sb.tile([128, QB], bf16)
        oc16, _, _ = offs16["cosb"]
        os16, _, _ = offs16["nsinb"]
        for c in range(n_cores):
            for off_, dst in ((oc16, cosk), (os16, nsink)):
                src = g16[c, off_ : off_ + 128 * QB].rearrange("(r w) -> r w", w=QB)
                nc.sync.dma_start(dst[:, c * QB : (c + 1) * QB], src)
        for off_, dst in ((oc16, cosq), (os16, nsinq)):
            src = pk16[off_ : off_ + 128 * QB].rearrange("(r w) -> r w", w=QB)
            nc.sync.dma_start(dst[:], src)

        negbig = sb.tile([128, 1], f32)
        nc.vector.memset(negbig[:], -BIG)

        # ------- x staging: f32 for indexer, cast to bf16 for main path -------
        tmpp = ctx.enter_context(tc.tile_pool(name="tmpp", bufs=3))
        xT16 = [sb.tile([128, T], bf16, name=f"xT16_{k}") for k in range(4)]
        xb16 = [sb.tile([128, QB], bf16, name=f"xb16_{k}") for k in range(4)]
        kiT = sb.tile([128, T], f32)  # duplicated halves
        qiT = [sb.tile([128, QB], f32, name=f"qiT{m}") for m in range(2)]
        wsig = [sb.tile([128, NIDX], f32, name=f"wsig{c}") for c in range(NCH)]
        oxb, _, _ = offs32["xTblk"]

        with tc.tile_pool(name="xf", bufs=2) as xf:
            xbf = []
            for k in range(4):
                t_ = xf.tile([128, QB], f32, tag=f"xbf{k}", name=f"xbf{k}", bufs=1)
                src = pk32[oxb + 128 * k * QB : oxb + 128 * (k + 1) * QB].rearrange(
                    "(r w) -> r w", w=QB
                )
                nc.sync.dma_start(t_[:], src)
                nc.vector.tensor_copy(xb16[k][:], t_[:])
                xbf.append(t_)

            for m in range(2):
                pq = ps.tile([128, 512], f32, tag="mm", name="qi_ps")
                for k in range(4):
                    nc.tensor.matmul(
                        pq[:, 0:QB],
                        Wiq_sb[k][:, 128 * m : 128 * (m + 1)],
                        xbf[k][:],
                        start=(k == 0),
                        stop=(k == 3),
                    )
                nc.scalar.copy(qiT[m][:], pq[:, 0:QB])

            for c in range(NCH):
                csl = slice(128 * c, 128 * (c + 1))
                pw = ps.tile([128, 512], f32, tag="mm", name="w_ps")
                for k in range(4):
                    nc.tensor.matmul(
                        pw[:, 0:NIDX], xbf[k][:, csl], Wiw_sb[k][:],
                        start=(k == 0), stop=(k == 3),
                    )
                nc.vector.tensor_tensor(
                    wsig[c][:], pw[:, 0:NIDX], biw_bc[:], op=ALU.add
                )
                nc.scalar.activation(wsig[c][:], wsig[c][:], AF.Sigmoid)

            for jb in range(NJB):
                jsl = slice(512 * jb, 512 * (jb + 1))
                xft = []
                for k in range(4):
                    t_ = xf.tile([128, 512], f32, tag="xft", name=f"xft{k}_{jb}")
                    for ci in range(CPJ):
                        c = (512 * jb) // QB + ci
                        src = g32[
                            c, oxb + 128 * k * QB : oxb + 128 * (k + 1) * QB
                        ].rearrange("(r w) -> r w", w=QB)
                        nc.sync.dma_start(t_[:, ci * QB : (ci + 1) * QB], src)
                    nc.vector.tensor_copy(xT16[k][:, jsl], t_[:])
                    xft.append(t_)
                pki = ps.tile([128, 512], f32, tag="mm", name="ki_ps")
                for k in range(4):
                    nc.tensor.matmul(
                        pki[0:64, :], Wik_sb[k][:], xft[k][:],
                        start=(k == 0), stop=(k == 3),
                    )
                nc.scalar.copy(kiT[0:64, jsl], pki[0:64, :])
                nc.sync.dma_start(kiT[64:128, jsl], kiT[0:64, jsl])

        # ---------------- projections (bf16 main path) ----------------
        def mm_acc(out_ps, lhsT_list, rhs_list):
            n = len(lhsT_list)
            for k in range(n):
                nc.tensor.matmul(
                    out_ps, lhsT_list[k], rhs_list[k],
                    start=(k == 0), stop=(k == n - 1),
                )

        def rope_apply(raw_bf, cos_t, nsin_t, out_bf, width):
            rot = ps.tile([128, 512], f32, tag="mm", name="rot_ps")
            nc.tensor.matmul(rot[:, 0:width], psgT[:], raw_bf, start=True, stop=True)
            t1 = tmpp.tile([128, 512], f32, tag="ropet", name="rope_t1")
            nc.vector.tensor_tensor(t1[:, 0:width], raw_bf, cos_t, op=ALU.mult)
            t2 = tmpp.tile([128, 512], f32, tag="ropet", name="rope_t2")
            nc.vector.tensor_tensor(
                t2[:, 0:width], rot[:, 0:width], nsin_t, op=ALU.mult
            )
            nc.vector.tensor_tensor(out_bf, t1[:, 0:width], t2[:, 0:width], op=ALU.add)

        kTr = [sb.tile([128, T], bf16, name=f"kTr{m}") for m in range(4)]
        for m in range(4):
            for jb in range(NJB):
                jsl = slice(512 * jb, 512 * (jb + 1))
                pk_ = ps.tile([128, 512], f32, tag="mm", name="kT_ps")
                mm_acc(
                    pk_[:],
                    [Wsb["Wk"][k][:, 128 * m : 128 * (m + 1)] for k in range(4)],
                    [xT16[k][:, jsl] for k in range(4)],
                )
                raw = tmpp.tile([128, 512], bf16, tag="raw16", name="kT_raw")
                nc.scalar.copy(raw[:], pk_[:])
                rope_apply(raw[:], cosk[:, jsl], nsink[:, jsl], kTr[m][:, jsl], 512)

        v_nat = [sb.tile([128, D], bf16, name=f"v{t}") for t in range(NT)]
        for t in range(NT):
            tsl = slice(128 * t, 128 * (t + 1))
            pv = ps.tile([128, 512], f32, tag="mm", name="v_ps")
            mm_acc(pv[:], [xT16[k][:, tsl] for k in range(4)],
                   [Wsb["Wv"][k][:] for k in range(4)])
            pg = ps.tile([128, 512], f32, tag="mm", name="vg_ps")
            mm_acc(pg[:], [xT16[k][:, tsl] for k in range(4)],
                   [Wsb["Wvg"][k][:] for k in range(4)])
            gs = tmpp.tile([128, D], f32, tag="vgs", name="vg_sig")
            nc.vector.tensor_tensor(gs[:], pg[:], bvg_bc[:], op=ALU.add)
            nc.scalar.activation(gs[:], gs[:], AF.Sigmoid)
            nc.vector.tensor_tensor(v_nat[t][:], pv[:], gs[:], op=ALU.mult)

        qTr = [sb.tile([128, QB], bf16, name=f"qTr{m}") for m in range(4)]
        for m in range(4):
            pq = ps.tile([128, 512], f32, tag="mm", name="qT_ps")
            mm_acc(pq[:, 0:QB],
                   [Wsb["Wq"][k][:, 128 * m : 128 * (m + 1)] for k in range(4)],
                   [xb16[k][:] for k in range(4)])
            raw = tmpp.tile([128, 512], bf16, tag="raw16", name="qT_raw")
            nc.scalar.activation(
                raw[:, 0:QB], pq[:, 0:QB], AF.Copy, scale=float(DH ** -0.5)
            )
            rope_apply(raw[:, 0:QB], cosq[:], nsinq[:], qTr[m][:], QB)

        ogT = [sb.tile([128, QB], bf16, name=f"ogT{m}") for m in range(4)]
        for m in range(4):
            pg = ps.tile([128, 512], f32, tag="mm", name="og_ps")
            mm_acc(pg[:, 0:QB],
                   [Wsb["Wog"][k][:, 128 * m : 128 * (m + 1)] for k in range(4)],
                   [xb16[k][:] for k in range(4)])
            nc.scalar.activation(
                ogT[m][:], pg[:, 0:QB], AF.Sigmoid, bias=bogT[m][:, 0:1]
            )

        # ---------------- indexer scores + selection ----------------
        fs = [sb.tile([128, T], f32, name=f"fs{c}") for c in range(NCH)]
        m01 = [sb.tile([128, T], f32, name=f"m01_{c}") for c in range(NCH)]
        for c in range(NCH):
            csl = slice(128 * c, 128 * (c + 1))
            for jb in range(NJB):
                jsl = slice(512 * jb, 512 * (jb + 1))
                for h in range(NIDX):
                    pr = ps.tile([128, 512], f32, tag="mm", name="raw_ps")
                    hr = slice(64 * (h % 2), 64 * (h % 2) + 64)
                    nc.tensor.matmul(
                        pr[:], qiT[h // 2][hr, csl], kiT[hr, jsl],
                        start=True, stop=True,
                    )
                    g = tmpp.tile([128, 512], f32, tag="gsig", name="g_sig")
                    nc.scalar.activation(
                        g[:], pr[:], AF.Sigmoid,
                        bias=idxb_bc[:, h : h + 1],
                        scale=float(1.0 / math.sqrt(DIDX)),
                    )
                    if h == 0:
                        nc.vector.tensor_scalar(
                            fs[c][:, jsl], g[:], wsig[c][:, 0:1], 1.0,
                            op0=ALU.mult, op1=ALU.add,
                        )
                    else:
                        nc.vector.scalar_tensor_tensor(
                            fs[c][:, jsl], in0=g[:], scalar=wsig[c][:, h : h + 1],
                            in1=fs[c][:, jsl], op0=ALU.mult, op1=ALU.add,
                        )
                nc.vector.scalar_tensor_tensor(
                    fs[c][:, jsl], in0=jramp[:, jsl], scalar=rowid[c][:, 0:1],
                    in1=fs[c][:, jsl], op0=ALU.is_le, op1=ALU.mult,
                )

            nlo = sb.tile([128, 1], f32, name=f"nlo{c}")
            nhi = sb.tile([128, 1], f32, name=f"nhi{c}")
            nc.vector.memset(nlo[:], -5.5)
            nc.vector.memset(nhi[:], -0.5)
            nmid = sb.tile([128, 1], f32, name=f"nmid{c}")
            S = sb.tile([128, 1], f32, name=f"S{c}")
            cfl = sb.tile([128, 1], f32, name=f"cfl{c}")
            cflm = sb.tile([128, 1], f32, name=f"cflm{c}")
            dd = sb.tile([128, 1], f32, name=f"dd{c}")
            thr = float(2 * KSEL - T)
            for it in range(N_BISECT):
                sgn_scr = tmpp.tile([128, T], bf16, tag="sgn", name="sgn_scr", bufs=1)
                nc.vector.tensor_tensor(nmid[:], nlo[:], nhi[:], op=ALU.add)
                nc.vector.tensor_scalar(nmid[:], nmid[:], 0.5, None, op0=ALU.mult)
                nc.scalar.activation(
                    sgn_scr[:], fs[c][:], AF.Sign, bias=nmid[:, 0:1], accum_out=S[:]
                )
                nc.vector.tensor_scalar(cfl[:], S[:], thr, None, op0=ALU.is_ge)
                nc.vector.tensor_scalar(
                    cflm[:], cfl[:], -1.0, 1.0, op0=ALU.mult, op1=ALU.add
                )
                nc.vector.tensor_tensor(dd[:], nmid[:], nhi[:], op=ALU.subtract)
                nc.vector.scalar_tensor_tensor(
                    nhi[:], in0=dd[:], scalar=cfl[:, 0:1], in1=nhi[:],
                    op0=ALU.mult, op1=ALU.add,
                )
                nc.vector.tensor_tensor(dd[:], nmid[:], nlo[:], op=ALU.subtract)
                nc.vector.scalar_tensor_tensor(
                    nlo[:], in0=dd[:], scalar=cflm[:, 0:1], in1=nlo[:],
                    op0=ALU.mult, op1=ALU.add,
                )
            tau = sb.tile([128, 1], f32, name=f"tau{c}")
            nc.vector.tensor_scalar(tau[:], nhi[:], -1.0, None, op0=ALU.mult)
            nc.vector.tensor_scalar(
                m01[c][:], fs[c][:], tau[:, 0:1], None, op0=ALU.is_ge
            )

        # ---------------- attention ----------------
        psT = ctx.enter_context(tc.tile_pool(name="psT", bufs=2, space="PSUM"))
        psA = ctx.enter_context(tc.tile_pool(name="psA", bufs=2, space="PSUM"))
        pp = ctx.enter_context(tc.tile_pool(name="pp", bufs=2))

        attn_g = []
        for hp in range(4):
            attn_ps = psA.tile([128, QB], f32, tag="attn", name=f"attn{hp}")
            for h in (2 * hp, 2 * hp + 1):
                hrow = slice(64 * (h % 2), 64 * (h % 2) + 64)
                pT = pp.tile(
                    [128, NT * NCH * 128], bf16, tag="pT", name=f"pT{h}", bufs=1
                )
                for c in range(NCH):
                    csl = slice(128 * c, 128 * (c + 1))
                    p_t = pp.tile([128, T], bf16, tag="p", name=f"p{h}_{c}")
                    den4 = pp.tile(
                        [128, NJB], f32, tag="den", name=f"den4_{h}_{c}", bufs=4
                    )
                    for jb in range(NJB):
                        jsl = slice(512 * jb, 512 * (jb + 1))
                        sps_ = ps.tile([128, 512], f32, tag="mm", name="s_ps")
                        nc.tensor.matmul(
                            sps_[:], qTr[h // 2][hrow, csl], kTr[h // 2][hrow, jsl],
                            start=True, stop=True,
                        )
                        nc.vector.scalar_tensor_tensor(
                            sps_[:], in0=m01[c][:, jsl], scalar=BIG, in1=sps_[:],
                            op0=ALU.mult, op1=ALU.add,
                        )
                        nc.scalar.activation(
                            p_t[:, jsl], sps_[:], AF.Exp,
                            bias=negbig[:, 0:1], accum_out=den4[:, jb : jb + 1],
                        )
                    den = pp.tile(
                        [128, 1], f32, tag="den1", name=f"den_{h}_{c}", bufs=4
                    )
                    nc.vector.reduce_sum(den[:], den4[:], axis=mybir.AxisListType.X)
                    inv = pp.tile(
                        [128, 1], f32, tag="inv", name=f"inv_{h}_{c}", bufs=4
                    )
                    nc.vector.reciprocal(inv[:], den[:])
                    nc.vector.tensor_scalar(
                        p_t[:], p_t[:], inv[:, 0:1], None, op0=ALU.mult
                    )
                    for t in range(NT):
                        trp = psT.tile([128, 128], bf16, tag="tr", name="tr_ps")
                        nc.tensor.transpose(
                            trp[:], p_t[:, 128 * t : 128 * (t + 1)], eye[:]
                        )
                        nc.vector.tensor_copy(
                            pT[:, t * NCH * 128 + 128 * c
                               : t * NCH * 128 + 128 * (c + 1)],
                            trp[:],
                        )
                for t in range(NT):
                    nc.tensor.matmul(
                        attn_ps[hrow, :],
                        v_nat[t][:, 64 * h : 64 * h + 64],
                        pT[:, t * NCH * 128 : (t + 1) * NCH * 128],
                        start=(t == 0),
                        stop=(t == NT - 1),
                    )
            ag = sb.tile([128, QB], bf16, name=f"attn_g{hp}")
            nc.vector.tensor_tensor(ag[:], attn_ps[:], ogT[hp][:], op=ALU.mult)
            attn_g.append(ag)

        for m in range(4):
            py = ps.tile([128, 512], f32, tag="mm", name="y_ps")
            mm_acc(py[:, 0:QB],
                   [Wsb["Wo"][k][:, 128 * m : 128 * (m + 1)] for k in range(4)],
                   [attn_g[k][:] for k in range(4)])
            ysb = tmpp.tile([128, QB], bf16, tag="ysb", name="y_sb")
            nc.scalar.copy(ysb[:], py[:, 0:QB])
            nc.sync.dma_start(yout[128 * m : 128 * (m + 1), :], ysb[:])

    return nc


def split_multi_waits(nc, max_waits=1):
    """Walrus here rejects instructions with more than one sync wait; split
    extras into single-wait NoOps on the same engine (in-order execution
    makes this semantics-preserving)."""
    n_extra = 0
    for f in nc.m.functions:
        for bb in f.blocks:
            insts = list(bb.instructions)
            out = []
            changed = False
            for inst in insts:
                si = inst.sync_info
                waits = list(si.on_wait) if si is not None else []
                if len(waits) > max_waits:
                    head, keep = waits[:-max_waits], waits[-max_waits:]
                    for w in head:
                        n_extra += 1
                        out.append(
                            mybir.InstNoOp(
                                name=f"I-waitsplit-{n_extra}",
                                engine=inst.engine,
                                sync_info=mybir.SyncInfo(on_wait=[w], on_update=[]),
                            )
                        )
                    inst.sync_info = mybir.SyncInfo(
                        on_wait=keep, on_update=list(si.on_update)
                    )
                    changed = True
                out.append(inst)
            if changed:
                bb.instructions = out
    return n_extra


def pack_inputs(inputs, T=T, n_cores=N_CORES):
    offs16, N16 = layout16(T, n_cores)
    offs32, N32 = layout32(T, n_cores)
    QB = T // n_cores
    x = np.asarray(inputs["x"], np.float32).reshape(T, D)
    xT = np.ascontiguousarray(x.T)
    rs = D // n_cores
    cosf, nsinf = make_rope_tables(T)
    cosf = cosf.astype(ml_dtypes.bfloat16)
    nsinf = nsinf.astype(ml_dtypes.bfloat16)

    in_maps = []
    for c in range(n_cores):
        pkb = np.empty(N16, dtype=ml_dtypes.bfloat16)
        for wn in W_NAMES:
            o, r, w = offs16[wn]
            pkb[o : o + r * w] = (
                np.asarray(inputs[wn], np.float32)[c * rs : (c + 1) * rs]
                .astype(ml_dtypes.bfloat16)
                .ravel()
            )
        o, r, w = offs16["cosb"]
        pkb[o : o + r * w] = cosf[:, c * QB : (c + 1) * QB].ravel()
        o, r, w = offs16["nsinb"]
        pkb[o : o + r * w] = nsinf[:, c * QB : (c + 1) * QB].ravel()

        pkf = np.empty(N32, dtype=np.float32)
        o, r, w = offs32["xTblk"]
        pkf[o : o + r * w] = xT[:, c * QB : (c + 1) * QB].ravel()
        for wn in ("Wiq", "Wik", "Wiw"):
            o, r, w = offs32[wn]
            pkf[o : o + r * w] = np.asarray(inputs[wn], np.float32)[
                c * rs : (c + 1) * rs
            ].ravel()

        aux = np.zeros(AUX_LEN, np.float32)
        aux[0:512] = np.asarray(inputs["bvg"], np.float32)
        aux[512:1024] = np.asarray(inputs["bog"], np.float32)
        aux[1024:1028] = np.asarray(inputs["biw"], np.float32)
        aux[1028:1032] = np.asarray(inputs["idx_bias"], np.float32)
        aux[1032] = float(c * QB)
        in_maps.append({"pk16": pkb, "pk32": pkf, "aux32": aux})
    return in_maps


def unpack_output(results, T=T, n_cores=N_CORES):
    QB = T // n_cores
    out = np.empty((T, D), np.float32)
    for c in range(n_cores):
        yT = np.asarray(results[c]["yout"]).astype(np.float32)
        out[c * QB : (c + 1) * QB, :] = yT.T
    return out.reshape(1, T, D)


def pack_concat(inputs, T=T, n_cores=N_CORES):
    """Pack inputs directly into the concatenated (n_cores*shape) arrays the
    sharded jit consumes (axis 0 = core)."""
    offs16, N16 = layout16(T, n_cores)
    offs32, N32 = layout32(T, n_cores)
    QB = T // n_cores
    x = np.asarray(inputs["x"], np.float32).reshape(T, D)
    xT = np.ascontiguousarray(x.T)
    rs = D // n_cores

    pkb = np.empty((n_cores, N16), dtype=ml_dtypes.bfloat16)
    pkf = np.empty((n_cores, N32), dtype=np.float32)
    aux = np.zeros((n_cores, AUX_LEN), np.float32)

    wb16 = {
        wn: np.asarray(inputs[wn], np.float32).astype(ml_dtypes.bfloat16)
        for wn in W_NAMES
    }
    for c in range(n_cores):
        for wn in W_NAMES:
            o, r, w = offs16[wn]
            pkb[c, o : o + r * w] = wb16[wn][c * rs : (c + 1) * rs].ravel()
        o, r, w = offs16["cosb"]
        pkb[c, o : o + r * w] = _COSB[:, c * QB : (c + 1) * QB].ravel()
        o, r, w = offs16["nsinb"]
        pkb[c, o : o + r * w] = _NSINB[:, c * QB : (c + 1) * QB].ravel()

        o, r, w = offs32["xTblk"]
        pkf[c, o : o + r * w] = xT[:, c * QB : (c + 1) * QB].ravel()
        for wn in ("Wiq", "Wik", "Wiw"):
            o, r, w = offs32[wn]
            pkf[c, o : o + r * w] = np.asarray(inputs[wn], np.float32)[
                c * rs : (c + 1) * rs
            ].ravel()

        aux[c, 0:512] = np.asarray(inputs["bvg"], np.float32)
        aux[c, 512:1024] = np.asarray(inputs["bog"], np.float32)
        aux[c, 1024:1028] = np.asarray(inputs["biw"], np.float32)
        aux[c, 1028:1032] = np.asarray(inputs["idx_bias"], np.float32)
        aux[c, 1032] = float(c * QB)
    return pkb.reshape(-1), pkf.reshape(-1), aux.reshape(-1)


def _make_runner(nc, n_cores=N_CORES):
    """Persistent jit for the SPMD NEFF (clone of run_bass_via_pjrt's
    multi-core branch with the jit hoisted out of the call path and the
    output buffer recycled as the next call's donated zero-arg)."""
    import jax
    from jax.sharding import Mesh, PartitionSpec
    from jax.experimental.shard_map import shard_map
    from concourse import bass2jax

    bass2jax.install_neuronx_cc_hook()
    assert nc.dbg_addr is None or not nc.dbg_callbacks

    partition_name = nc.partition_id_tensor.name if nc.partition_id_tensor else None
    in_names, out_names, out_avals, out_shapes = [], [], [], []
    for alloc in nc.m.functions[0].allocations:
        if not isinstance(alloc, mybir.MemoryLocationSet):
            continue
        name = alloc.memorylocations[0].name
        if alloc.kind == "ExternalInput":
            if name != partition_name:
                in_names.append(name)
        elif alloc.kind == "ExternalOutput":
            shape = tuple(alloc.tensor_shape)
            dtype = mybir.dt.np(alloc.dtype)
            out_names.append(name)
            out_avals.append(jax.core.ShapedArray(shape, dtype))
            out_shapes.append((shape, dtype))
    n_params = len(in_names)
    n_outs = len(out_avals)
    all_in = list(in_names) + list(out_names)
    if partition_name is not None:
        all_in.append(partition_name)
    donate = tuple(range(n_params, n_params + n_outs))

    def _body(*args):
        operands = list(args)
        if partition_name is not None:
            operands.append(bass2jax.partition_id_tensor())
        outs = bass2jax._bass_exec_p.bind(
            *operands,
            out_avals=tuple(out_avals),
            in_names=tuple(all_in),
            out_names=tuple(out_names),
            lowering_input_output_aliases=(),
            sim_require_finite=True,
            sim_require_nnan=True,
            nc=nc,
        )
        return tuple(outs)

    devices = jax.devices()[:n_cores]
    mesh = Mesh(np.asarray(devices), ("core",))
    in_specs = (PartitionSpec("core"),) * (n_params + n_outs)
    out_specs = (PartitionSpec("core"),) * n_outs
    sharded = jax.jit(
        shard_map(_body, mesh=mesh, in_specs=in_specs, out_specs=out_specs,
                  check_rep=False),
        donate_argnums=donate,
        keep_unused=True,
    )

    state = {"prev_out": None}

    def run(concat_inputs):
        if state["prev_out"] is None:
            zouts = [
                np.zeros((n_cores * s[0], *s[1:]), dt) for s, dt in out_shapes
            ]
        else:
            zouts = state["prev_out"]
        out_arrs = sharded(*concat_inputs, *zouts)
        state["prev_out"] = list(out_arrs)
        return [np.asarray(a) for a in out_arrs]

    return run, in_names, out_names


# ---------------- import-time build + compile + warm-up ----------------
_COSB, _NSINB = (t.astype(ml_dtypes.bfloat16) for t in make_rope_tables(T))
_nc = build_nc()
split_multi_waits(_nc)
_run, _IN_NAMES, _OUT_NAMES = _make_runner(_nc)
assert _IN_NAMES == ["pk16", "pk32", "aux32"] and _OUT_NAMES == ["yout"]

_dummy = {
    "x": np.zeros((1, T, D), np.float32),
    **{wn: np.zeros((D, D), np.float32) for wn in W_NAMES},
    "Wiq": np.zeros((D, NIDX * DIDX), np.float32),
    "Wik": np.zeros((D, DIDX), np.float32),
    "Wiw": np.zeros((D, NIDX), np.float32),
    "biw": np.zeros((NIDX,), np.float32),
    "idx_bias": np.zeros((NIDX,), np.float32),
    "bvg": np.zeros((D,), np.float32),
    "bog": np.zeros((D,), np.float32),
}
for _ in range(2):
    _run(pack_concat(_dummy))


def kernel(**inputs):
    pkb, pkf, aux = pack_concat(inputs)
    outs = _run((pkb, pkf, aux))
    QB = T // N_CORES
    yT = outs[0].reshape(N_CORES, D, QB).astype(np.float32)
    out = np.empty((T, D), np.float32)
    for c in range(N_CORES):
        out[c * QB : (c + 1) * QB, :] = yT[c].T
    return out.reshape(1, T, D)
